# revision 18
# baseline (speedup 1.0000x reference)
"""Trainium2 Bass kernel for nn_AbsoluteAttention (B=2,T=2048,DIM=1024,H=16,DH=64,DT=32).

Key algebraic restructuring (exact in real arithmetic):
  - sum(softmax(q), -1) == 1, so the whole Q path collapses:
    q_attentioned[b,t,l,h] = sum_d t_q[t,h,d] * t_k[l,h,d]  (rank-64, data-independent)
  - loading[b,:,h,:] = t_q[:,h,:] @ (t_k[:,h,:].T @ kv[b,:,h,:])  (associativity)
    -> no [T,T] attention matrix is ever materialized.
  - RMSNorm scale rnorm[t] is folded into the softmax logits scale (ACT 'scale')
    and the kv normalization (1/s * rnorm), ln_w folded into Wk/Wv on host.

Sharding: 8 cores = (batch b in {0,1}) x (head-group hg in {0..3}, 4 heads each).
Each core computes a partial output [T, DIM] = loading_hg @ Wo_hg.T for its batch;
host sums the 4 head-group partials per batch.  No collectives.
"""
import os
import numpy as np

# The axon NTFF trace hook module (antenv.axon_hooks) is absent in this
# container; make sure run_bass_kernel_spmd never takes the trace path.
os.environ.setdefault("BASS_NEVER_TRACE", "1")

try:
    import concourse.bass as bass  # noqa
except ImportError:
    import sys
    for p in ('/opt/trn_rl_repo', '/root/.axon_site/_ro/trn_rl_repo'):
        sys.path.append(p)

import concourse.bacc as bacc
import concourse.mybir as mybir
from concourse.tile import TileContext
from concourse import bass_utils

B, T, DIM, H, DH, DT = 2, 2048, 1024, 16, 64, 32
EPS = float(np.finfo(np.float32).eps)
INV_SQRT_DH = np.float32(1.0 / np.sqrt(DH))
HG = 4            # head-groups
HPG = H // HG     # heads per group (4)
JG = HPG * DH     # j-dims per group (256)
NT = T // 128     # 16 token tiles
F32 = mybir.dt.float32
F32R = mybir.dt.float32r
F16 = mybir.dt.float16
EXP = mybir.ActivationFunctionType.Exp
AX = mybir.AxisListType.X


def build_nc(niter=1, p1=True, p2=True, p3=True, din=True):
    nc = bacc.Bacc("TRN2", target_bir_lowering=False, debug=False)
    sT_d = nc.dram_tensor("sT", [DIM, T], F16, kind="ExternalInput").ap()
    wkvT_d = nc.dram_tensor("wkvT", [DIM, 2 * JG], F16, kind="ExternalInput").ap()
    woT_d = nc.dram_tensor("woT", [JG, DIM], F32R, kind="ExternalInput").ap()
    tqT_d = nc.dram_tensor("tqT", [JG, T], F32R, kind="ExternalInput").ap()
    tk_d = nc.dram_tensor("tk", [T, 2 * DT], F32R, kind="ExternalInput").ap()
    sc_d = nc.dram_tensor("sc", [T], F32, kind="ExternalInput").ap()
    out_d = nc.dram_tensor("out", [T, DIM], F32, kind="ExternalOutput").ap()

    with TileContext(nc) as tc:
        with tc.tile_pool(name="singles", bufs=1) as singles, \
             tc.tile_pool(name="stp", bufs=3) as stp, \
             tc.tile_pool(name="work", bufs=3) as work, \
             tc.tile_pool(name="small", bufs=4) as small, \
             tc.tile_pool(name="outp", bufs=8) as outp, \
             tc.tile_pool(name="ppk", bufs=2, space="PSUM") as ppk, \
             tc.tile_pool(name="ppv", bufs=2, space="PSUM") as ppv, \
             tc.tile_pool(name="ppS", bufs=1, space="PSUM") as ppS, \
             tc.tile_pool(name="pp3", bufs=3, space="PSUM") as pp3:

            # ---- weights / constants ----
            # first token-group DMA + wk/wv go FIRST so PE can start ASAP;
            # wo/tq/tk/sc are only needed in later phases.
            GROUPS = [(0, 1), (1, 3), (4, 4), (8, 4), (12, 4)]  # (start_tile, ntiles)
            st_g0 = stp.tile([128, 8, 128], F16, name="st_pre", tag="st0")
            nc.sync.dma_start(
                out=st_g0,
                in_=sT_d.rearrange("(c p) t -> p c t", p=128)[:, :, 0:128])
            wkv_s = singles.tile([128, 8, 2 * JG], F16)
            nc.sync.dma_start(out=wkv_s,
                              in_=wkvT_d.rearrange("(c p) n -> p c n", p=128))
            sc_s = singles.tile([128, NT], F32)
            nc.sync.dma_start(out=sc_s, in_=sc_d.rearrange("(n p) -> p n", p=128))
            tk_s = singles.tile([128, NT, 2 * DT], F32R)
            nc.sync.dma_start(out=tk_s, in_=tk_d.rearrange("(n p) d -> p n d", p=128))
            wo_s = singles.tile([128, 2, DIM], F32R)
            tq_s = singles.tile([64, HPG, T], F32R)

            for it in range(niter):
                # ---- phase 1: projections + softmax + kv + S accumulation ----
                ps_S = ppS.tile([64, JG], F32, name=f"ps_S_{it}", tag="ps_S")
                ev_all = singles.tile([128, NT, JG], F32R, name=f"ev_all_{it}",
                                      tag="ev_all")

                def emit_s_mm(i):
                    # S for all 4 heads in one matmul: lhsT tk is head-independent
                    nc.tensor.matmul(ps_S, tk_s[:, i, :], ev_all[:, i, :],
                                     start=(i == 0), stop=(i == NT - 1))
                for g, (t0, ntl) in enumerate(GROUPS):
                    if it == 0 and g == 0:
                        st_g = st_g0
                    else:
                        st_g = stp.tile([128, 8, ntl * 128], F16,
                                        name=f"st_{it}_{g}",
                                        tag="st0" if ntl == 1 else "st")
                        nc.sync.dma_start(
                            out=st_g,
                            in_=sT_d.rearrange("(c p) t -> p c t", p=128)
                                    [:, :, t0 * 128:(t0 + ntl) * 128])
                    if it == 0 and g == 3:
                        # phase-2/3 weights: after the bulk of phase-1 streaming
                        nc.sync.dma_start(
                            out=wo_s, in_=woT_d.rearrange("(c p) n -> p c n", p=128))
                        nc.sync.dma_start(
                            out=tq_s, in_=tqT_d.rearrange("(h p) n -> p h n", p=64))
                    for il in range(ntl):
                        if not p1:
                            break
                        i = t0 + il
                        tsl = slice(il * 128, (il + 1) * 128)
                        psk = ppk.tile([128, JG], F32, name=f"psk_{it}_{i}", tag="psk")
                        psv = ppv.tile([128, JG], F32, name=f"psv_{it}_{i}", tag="psv")
                        for c in range(8):
                            nc.tensor.matmul(psk, st_g[:, c, tsl],
                                             wkv_s[:, c, 0:JG],
                                             start=(c == 0), stop=(c == 7))
                        for c in range(8):
                            nc.tensor.matmul(psv, st_g[:, c, tsl],
                                             wkv_s[:, c, JG:2 * JG],
                                             start=(c == 0), stop=(c == 7))
                        e_t = work.tile([128, JG], F32, tag="e")
                        nc.scalar.activation(out=e_t, in_=psk, func=EXP,
                                             scale=sc_s[:, i:i + 1])
                        ssum = small.tile([128, HPG], F32, tag="ssum")
                        nc.vector.reduce_sum(
                            out=ssum.rearrange("p (f o) -> p f o", o=1),
                            in_=e_t.rearrange("p (h d) -> p h d", h=HPG), axis=AX)
                        ev2 = work.tile([128, JG], F32, tag="ev2")
                        nc.vector.tensor_mul(ev2, e_t, psv)
                        rec = small.tile([128, HPG], F32, tag="rec")
                        nc.vector.reciprocal(rec, ssum)
                        nc.vector.tensor_mul(
                            ev_all[:, i, :].rearrange("p (h d) -> p h d", h=HPG),
                            ev2.rearrange("p (h d) -> p h d", h=HPG),
                            rec.rearrange("p (h o) -> p h o", o=1)
                               .broadcast_to((128, HPG, DH)))
                        if i >= 2:
                            emit_s_mm(i - 2)
                for i in range(NT - 2, NT):
                    if p1:
                        emit_s_mm(i)

                # ---- phase 2: loading_T = S^T-style matmul into lt_s ----
                S_sb = singles.tile([64, HPG, DH], F32R, name=f"S_sb_{it}", tag="S_sb")
                if p1:
                    nc.vector.tensor_copy(S_sb.rearrange("p h d -> p (h d)"), ps_S)
                else:
                    nc.vector.memset(S_sb.rearrange("p h d -> p (h d)").bitcast(F32), 0.0)
                lt_s = singles.tile([128, 2, T], F32R, name=f"lt_{it}", tag="lt")
                for q in range(4 if p2 else 0):      # T/512 chunks
                    for h in range(HPG):
                        psl = pp3.tile([64, 512], F32, name=f"psl_{it}_{h}_{q}",
                                       tag="p3")
                        nc.tensor.matmul(psl, S_sb[:, h, :],
                                         tq_s[:, h, q * 512:(q + 1) * 512],
                                         start=True, stop=True)
                        dst = lt_s[(h % 2) * 64:(h % 2) * 64 + 64, h // 2,
                                   q * 512:(q + 1) * 512]
                        if h % 2 == 0:
                            nc.vector.tensor_copy(dst, psl)
                        else:
                            nc.scalar.copy(dst, psl)

                # ---- phase 3: partial out = loading @ Wo_hg^T ----
                if not p2:
                    nc.vector.memset(lt_s[:, 0, :].bitcast(F32), 0.0)
                    nc.vector.memset(lt_s[:, 1, :].bitcast(F32), 0.0)
                for i in range(NT if p3 else 0):
                    tsl = slice(i * 128, (i + 1) * 128)
                    for n2 in range(2):
                        nsl = slice(n2 * 512, (n2 + 1) * 512)
                        pso = pp3.tile([128, 512], F32, name=f"pso_{it}_{i}_{n2}",
                                       tag="p3")
                        for kc in range(2):
                            nc.tensor.matmul(
                                pso, lt_s[:, kc, tsl], wo_s[:, kc, nsl],
                                start=(kc == 0), stop=(kc == 1))
                        out_s = outp.tile([128, 512], F32, tag="out_s")
                        if (2 * i + n2) % 2 == 1:
                            nc.scalar.copy(out_s, pso)
                        else:
                            nc.vector.tensor_copy(out_s, pso)
                        nc.sync.dma_start(out=out_d[tsl, nsl], in_=out_s)

    nc.compile()
    return nc


def host_prep(inputs):
    """Returns per-core in_maps (list of 8 dicts)."""
    states = np.asarray(inputs["states"], np.float32)
    mask = np.asarray(inputs["attention_mask"])
    ln_w = np.asarray(inputs["ln_w"], np.float32)
    time_angles = np.asarray(inputs["time_angles"], np.float32)
    head_time_delta = np.asarray(inputs["head_time_delta"], np.float32)
    Wk = np.asarray(inputs["Wk"], np.float32)
    Wv = np.asarray(inputs["Wv"], np.float32)
    Wo = np.asarray(inputs["Wo"], np.float32)
    for nm in ("bk", "bv", "bo"):
        assert not np.asarray(inputs[nm]).any(), f"{nm} must be zero"

    rnorm = 1.0 / np.sqrt(np.mean(states.astype(np.float64) ** 2, axis=-1) + EPS)
    rnorm = rnorm.astype(np.float32)                     # [B,T]
    scale = (rnorm * mask.astype(np.float32))            # [B,T]

    Wk2 = (Wk * ln_w[None, :]).astype(np.float32)
    Wv2 = (Wv * ln_w[None, :]).astype(np.float32)

    # time embeddings, ang in strict fp32 like the reference
    pos = np.arange(T, dtype=np.float32)[:, None, None]            # [T,1,1]
    pos_q = (pos + head_time_delta[None, :, None]).astype(np.float32)  # [T,H,1]
    ang_q = (pos_q * time_angles).astype(np.float32)               # [T,H,DT]
    cq, sq = np.cos(ang_q), np.sin(ang_q)
    tq = (np.concatenate([cq + sq, cq - sq], -1) * INV_SQRT_DH).astype(np.float32)
    ang_k = (pos[:, 0, :] * time_angles).astype(np.float32)        # [T,DT]
    ck, sk = np.cos(ang_k), np.sin(ang_k)
    tk = (np.concatenate([ck + sk, ck - sk], -1) * INV_SQRT_DH).astype(np.float32)

    sT = [np.ascontiguousarray(states[b].T) for b in range(B)]     # [DIM,T]
    in_maps = []
    for core in range(8):
        b, hg = core // HG, core % HG
        jsl = slice(hg * JG, (hg + 1) * JG)
        hsl = slice(hg * HPG, (hg + 1) * HPG)
        in_maps.append(dict(
            sT=sT[b].astype(np.float16),
            wkvT=np.concatenate([Wk2[jsl, :].T, Wv2[jsl, :].T],
                                axis=1).astype(np.float16),
            woT=np.ascontiguousarray(Wo[:, jsl].T),
            tqT=np.ascontiguousarray(
                tq[:, hsl, :].transpose(1, 2, 0).reshape(JG, T)),
            tk=(tk * rnorm[b][:, None]).astype(np.float32),
            sc=np.ascontiguousarray(scale[b]),
        ))
    return in_maps


def gather(results, bo):
    out = np.zeros((B, T, DIM), np.float32)
    for core in range(8):
        out[core // HG] += results[core]["out"]
    if bo.any():
        out += bo[None, None, :]
    return out


class _SpmdRunner:
    """Compile-once PJRT runner (repeat kernel() calls skip retrace/recompile)."""

    def __init__(self, nc, n_cores=8):
        import jax
        from jax.sharding import Mesh, PartitionSpec
        from jax.experimental.shard_map import shard_map
        from concourse.bass2jax import (
            _bass_exec_p, install_neuronx_cc_hook, partition_id_tensor)
        install_neuronx_cc_hook()
        self.jax = jax
        self.n_cores = n_cores
        in_names, out_names, out_avals, zero_outs = [], [], [], []
        for alloc in nc.m.functions[0].allocations:
            if not isinstance(alloc, mybir.MemoryLocationSet):
                continue
            name = alloc.memorylocations[0].name
            if alloc.kind == "ExternalInput":
                in_names.append(name)
            elif alloc.kind == "ExternalOutput":
                out_names.append(name)
                shape = tuple(alloc.tensor_shape)
                dtype = mybir.dt.np(alloc.dtype)
                out_avals.append(jax.core.ShapedArray(shape, dtype))
                zero_outs.append(np.zeros(shape, dtype))
        pname = nc.partition_id_tensor.name if nc.partition_id_tensor else None
        in_names = [n for n in in_names if n != pname]
        self.in_names, self.out_names = in_names, out_names
        self.out_avals, self.zero_outs = out_avals, zero_outs
        n_params = len(in_names)
        all_in_names = tuple(in_names + out_names)
        if pname is not None:
            all_in_names = all_in_names + (pname,)

        def _body(*args):
            operands = list(args)
            if pname is not None:
                operands.append(partition_id_tensor())
            outs = _bass_exec_p.bind(
                *operands,
                out_avals=tuple(out_avals),
                in_names=all_in_names,
                out_names=tuple(out_names),
                lowering_input_output_aliases=(),
                sim_require_finite=True,
                sim_require_nnan=True,
                nc=nc,
            )
            return tuple(outs)

        devices = jax.devices()[:n_cores]
        self.mesh = Mesh(np.asarray(devices), ("core",))
        specs = (PartitionSpec("core"),) * (n_params + len(out_names))
        self.fn = jax.jit(
            shard_map(_body, mesh=self.mesh, in_specs=specs,
                      out_specs=(PartitionSpec("core"),) * len(out_names),
                      check_rep=False),
            keep_unused=True,
        )

    def run(self, in_maps):
        jax = self.jax
        from jax.sharding import NamedSharding, PartitionSpec
        sharding = NamedSharding(self.mesh, PartitionSpec("core"))
        args = []
        for name in self.in_names:
            cat = np.concatenate([np.asarray(m[name]) for m in in_maps], axis=0)
            args.append(jax.device_put(cat, sharding))
        for z in self.zero_outs:
            cat = np.zeros((self.n_cores * z.shape[0], *z.shape[1:]), z.dtype)
            args.append(jax.device_put(cat, sharding))
        outs = self.fn(*args)
        jax.block_until_ready(outs)
        results = []
        for c in range(self.n_cores):
            results.append({
                name: np.asarray(outs[i]).reshape(
                    self.n_cores, *self.out_avals[i].shape)[c]
                for i, name in enumerate(self.out_names)})
        return results


_CACHE = {}


def kernel(**inputs) -> np.ndarray:
    in_maps = host_prep(inputs)
    try:
        if "runner" not in _CACHE:
            _CACHE["runner"] = _SpmdRunner(build_nc())
        results = _CACHE["runner"].run(in_maps)
    except Exception:
        # Fallback: the blessed (but retrace-per-call) execution path.
        if "nc" not in _CACHE:
            _CACHE["nc"] = build_nc()
        results = bass_utils.run_bass_kernel_spmd(
            _CACHE["nc"], in_maps, core_ids=list(range(8))).results
    return gather(results, np.asarray(inputs["bo"], np.float32))
